# revision 10
# baseline (speedup 1.0000x reference)
"""Bahdanau-style additive attention kernel for Trainium2 (8 NeuronCores).

reference:
    q_h    = relu(query @ w1.T)                      (B, H)
    k_h    = relu(key @ w2.T)                        (B, T, H)
    scores = tanh(q_h[:, None, :] + k_h) @ w_out     (B, T)
    attn   = softmax(scores, axis=1)
    out    = einsum('bt,bth->bh', attn, key)         (B, H)

B=128, T=2048, H=512. Pure data parallel: 16 batch rows per core.

Device strategy (per core):
  pass 1: k_h matmul contracts over h, so key needs h on partitions.  The
  host pre-transposes key -> keyT [b, h, t] (cast bf16) so the device reads
  it with a natural contiguous DMA.  psum tile layout [g_sub=128, t=512];
  tanh(k_h + q_h) is one ScalarE activation reading PSUM with the per-
  partition bias = q_h (computed on device, transposed layout [g, b]).
  scores = sum_g w_out[g] * tanh(...) is a M=1 matmul (lhsT = w_out block).
  pass 2: softmax runs partition-parallel on [8, T] groups; out = attn @ key
  contracts over t, using natural-layout fp32 key tiles as the moving
  operand (float32r: full PE rate at N=512) and attn columns as weights.

NOTE (reference relu bug): reference applies relu to BOTH q_h and k_h.
"""

import numpy as np
import ml_dtypes

import concourse.bass as bass
from concourse import bacc
import concourse.mybir as mybir
import concourse.tile as tile
from concourse import bass_utils

B, T, H = 128, 2048, 512
NCORES = 8
BPC = B // NCORES          # 16 batch rows per core
P = 128
HS = H // P                # 4 subtiles of the h/g dims
TC = 512                   # pass-1 t-chunk (max fp32 psum free dim)
NCH = T // TC              # 4 chunks
GROUP = 8                  # batch rows per softmax group
TP = T // P                # 16 t-subtiles for pass 2

f32 = mybir.dt.float32
f32r = mybir.dt.float32r
bf16 = mybir.dt.bfloat16
AF = mybir.ActivationFunctionType
AX = mybir.AxisListType

_CACHE = {}


def _build_nc():
    nc = bacc.Bacc(trn_type="TRN2", target_bir_lowering=False)

    keyT_bf = nc.dram_tensor("keyT_bf", [BPC, H, T], bf16, kind="ExternalInput")
    key_f32 = nc.dram_tensor("key_f32", [BPC, T, H], f32, kind="ExternalInput")
    w2T_bf = nc.dram_tensor("w2T_bf", [H, H], bf16, kind="ExternalInput")
    w1T = nc.dram_tensor("w1T", [H, H], f32, kind="ExternalInput")
    qT = nc.dram_tensor("qT", [H, BPC], f32, kind="ExternalInput")
    wout = nc.dram_tensor("wout", [H, 1], bf16, kind="ExternalInput")
    out = nc.dram_tensor("out", [BPC, H], f32, kind="ExternalOutput")

    with tile.TileContext(nc) as tc:
        with (
            tc.tile_pool(name="const", bufs=1) as cpool,
            tc.tile_pool(name="keyT", bufs=3) as keyT_pool,
            tc.tile_pool(name="keynat", bufs=8) as keynat_pool,
            tc.tile_pool(name="tanh", bufs=3) as tanh_pool,
            tc.tile_pool(name="sm", bufs=2) as sm_pool,
            tc.tile_pool(name="dram", bufs=2, space="DRAM") as dram_pool,
            tc.tile_pool(name="ps_kh", bufs=3, space="PSUM") as ps_kh,
            tc.tile_pool(name="ps_sc", bufs=3, space="PSUM") as ps_sc,
            tc.tile_pool(name="ps_o", bufs=2, space="PSUM") as ps_o,
        ):
            # ---- constants ----
            w2T_sb = cpool.tile([P, HS, H], bf16)      # [p, h_sub, g]
            nc.sync.dma_start(w2T_sb[:], w2T_bf.ap().rearrange("(s p) g -> p s g", p=P))
            w1T_sb = cpool.tile([P, HS, H], f32)
            nc.sync.dma_start(w1T_sb[:], w1T.ap().rearrange("(s p) g -> p s g", p=P))
            qT_sb = cpool.tile([P, HS, BPC], f32)
            nc.sync.dma_start(qT_sb[:], qT.ap().rearrange("(s p) b -> p s b", p=P))
            wout_sb = cpool.tile([P, HS], bf16)        # [p, g_sub]; lhsT slice [:, gs:gs+1]
            nc.sync.dma_start(wout_sb[:], wout.ap().rearrange("(s p) o -> p (s o)", p=P))
            qhT_sb = cpool.tile([P, HS, BPC], f32)     # relu(w1 q) with g on partitions

            # ---- q_h (transposed layout [g, b]) ----
            for gs in range(HS):
                ps_q = ps_kh.tile([P, TC], f32, tag="kh")
                for hs in range(HS):
                    nc.tensor.matmul(
                        ps_q[:, :BPC],
                        lhsT=w1T_sb[:, hs, gs * P:(gs + 1) * P],
                        rhs=qT_sb[:, hs, :],
                        start=(hs == 0),
                        stop=(hs == HS - 1),
                    )
                nc.scalar.activation(qhT_sb[:, gs, :], ps_q[:, :BPC], AF.Relu)

            # ---- main: groups of GROUP batch rows ----
            for g in range(BPC // GROUP):
                scores_sb = sm_pool.tile([GROUP, T], f32, tag="scores")

                # pass 1: scores
                for j in range(GROUP):
                    b = g * GROUP + j
                    keyT_b = keyT_bf.ap()[b].rearrange("(s p) t -> p s t", p=P)
                    stage = tanh_pool.tile([1, T], f32, tag="stage")
                    for c in range(NCH):
                        kt = keyT_pool.tile([P, HS, TC], bf16)
                        nc.sync.dma_start(kt[:], keyT_b[:, :, c * TC:(c + 1) * TC])
                        ps_scores = ps_sc.tile([1, TC], f32)
                        for gs in range(HS):
                            ps_t = ps_kh.tile([P, TC], f32, tag="kh")
                            for hs in range(HS):
                                nc.tensor.matmul(
                                    ps_t[:],
                                    lhsT=w2T_sb[:, hs, gs * P:(gs + 1) * P],
                                    rhs=kt[:, hs, :],
                                    start=(hs == 0),
                                    stop=(hs == HS - 1),
                                )
                            # tanh(relu(k_h) + q_h): relu in-place on DVE,
                            # then tanh + per-partition q_h bias on ScalarE
                            rl = tanh_pool.tile([P, TC], f32, tag="rl")
                            nc.vector.tensor_scalar_max(rl[:], ps_t[:], 0.0)
                            th = tanh_pool.tile([P, TC], bf16)
                            nc.scalar.activation(
                                th[:], rl[:], AF.Tanh,
                                bias=qhT_sb[:, gs, b:b + 1],
                            )
                            nc.tensor.matmul(
                                ps_scores[:],
                                lhsT=wout_sb[:, gs:gs + 1],
                                rhs=th[:],
                                start=(gs == 0),
                                stop=(gs == HS - 1),
                            )
                        nc.vector.tensor_copy(
                            stage[:, c * TC:(c + 1) * TC], ps_scores[:]
                        )
                    nc.sync.dma_start(scores_sb[j:j + 1, :], stage[:])

                # softmax over T, partition-parallel across the group
                mx = sm_pool.tile([GROUP, 1], f32, tag="mx")
                nc.vector.reduce_max(mx[:], scores_sb[:], axis=AX.X)
                nmx = sm_pool.tile([GROUP, 1], f32, tag="nmx")
                nc.vector.tensor_scalar_mul(nmx[:], mx[:], -1.0)
                expv = sm_pool.tile([GROUP, T], f32, tag="expv")
                sums = sm_pool.tile([GROUP, 1], f32, tag="sums")
                nc.scalar.activation(
                    expv[:], scores_sb[:], AF.Exp, bias=nmx[:], accum_out=sums[:]
                )
                inv = sm_pool.tile([GROUP, 1], f32, tag="inv")
                nc.vector.reciprocal(inv[:], sums[:])
                attn = sm_pool.tile([GROUP, T], f32, tag="attn")
                nc.vector.tensor_scalar_mul(attn[:], expv[:], inv[:])

                # transpose attn via DRAM round-trip: [GROUP, T] -> [P, TP*GROUP]
                attn_dram = dram_pool.tile([GROUP, T], f32)
                nc.sync.dma_start(attn_dram[:], attn[:])
                attnT = sm_pool.tile([P, TP, GROUP], f32, tag="attnT")
                for j in range(GROUP):
                    nc.sync.dma_start(
                        attnT[:, :, j:j + 1],
                        attn_dram[j].rearrange("(c p) -> p c", p=P)[:, :, None],
                    )

                # pass 2: out = attn @ key (contract over t)
                for j in range(GROUP):
                    b = g * GROUP + j
                    ps_out = ps_o.tile([1, H], f32)
                    for c2 in range(TP):
                        kn = keynat_pool.tile([P, H], f32)
                        nc.sync.dma_start(kn[:], key_f32.ap()[b, c2 * P:(c2 + 1) * P, :])
                        nc.tensor.matmul(
                            ps_out[:],
                            lhsT=attnT[:, c2, j:j + 1],
                            rhs=kn[:],
                            start=(c2 == 0),
                            stop=(c2 == TP - 1),
                        )
                    ostage = tanh_pool.tile([1, H], f32, tag="ostage")
                    nc.vector.tensor_copy(ostage[:], ps_out[:])
                    nc.sync.dma_start(out.ap()[b:b + 1, :], ostage[:])

    nc.compile()
    return nc


def kernel(query, key, w1, w2, w_out):
    query = np.asarray(query, dtype=np.float32)
    key = np.asarray(key, dtype=np.float32)
    w1 = np.asarray(w1, dtype=np.float32)
    w2 = np.asarray(w2, dtype=np.float32)
    w_out = np.asarray(w_out, dtype=np.float32)

    if "nc" not in _CACHE:
        _CACHE["nc"] = _build_nc()
    nc = _CACHE["nc"]

    w2T_bf = np.ascontiguousarray(w2.T).astype(ml_dtypes.bfloat16)
    w1T = np.ascontiguousarray(w1.T)
    wout_col = np.ascontiguousarray(w_out.reshape(H, 1)).astype(ml_dtypes.bfloat16)
    qT = np.ascontiguousarray(query.T)  # [H, B]

    in_maps = []
    for c in range(NCORES):
        sl = slice(c * BPC, (c + 1) * BPC)
        key_c = key[sl]
        in_maps.append({
            "keyT_bf": np.ascontiguousarray(
                key_c.transpose(0, 2, 1)).astype(ml_dtypes.bfloat16),
            "key_f32": np.ascontiguousarray(key_c),
            "w2T_bf": w2T_bf,
            "w1T": w1T,
            "qT": np.ascontiguousarray(qT[:, sl]),
            "wout": wout_col,
        })

    _CACHE["in_maps"] = in_maps
    import os
    trace = bool(int(os.environ.get("BENCH_TRACE", "0")))
    res = bass_utils.run_bass_kernel_spmd(
        nc, in_maps, core_ids=list(range(NCORES)), trace=trace)
    if trace:
        print(f"HW exec time: {res.exec_time_ns} ns")
        if res.instructions_and_trace:
            print("trace:", res.instructions_and_trace[1])
        _CACHE["res"] = res
    out = np.concatenate([r["out"] for r in res.results], axis=0)
    return out.astype(np.float32)


# revision 15
# speedup vs baseline: 141.2960x; 141.2960x over previous
"""Bahdanau-style additive attention kernel for Trainium2 (8 NeuronCores).

reference:
    q_h    = relu(query @ w1.T)                      (B, H)
    k_h    = relu(key @ w2.T)                        (B, T, H)
    scores = tanh(q_h[:, None, :] + k_h) @ w_out     (B, T)
    attn   = softmax(scores, axis=1)
    out    = einsum('bt,bth->bh', attn, key)         (B, H)

B=128, T=2048, H=512. Pure data parallel: 16 batch rows per core.

Device strategy (per core):
  pass 1: k_h matmul contracts over h, so key needs h on partitions.  The
  host pre-transposes key -> keyT [b, h, t] (cast bf16) so the device reads
  it with a natural contiguous DMA.  psum tile layout [g_sub=128, t=512];
  tanh(k_h + q_h) is one ScalarE activation reading PSUM with the per-
  partition bias = q_h (computed on device, transposed layout [g, b]).
  scores = sum_g w_out[g] * tanh(...) is a M=1 matmul (lhsT = w_out block).
  pass 2: softmax runs partition-parallel on [8, T] groups; out = attn @ key
  contracts over t, using natural-layout fp32 key tiles as the moving
  operand (float32r: full PE rate at N=512) and attn columns as weights.

NOTE (reference relu bug): reference applies relu to BOTH q_h and k_h.
"""

import numpy as np
import ml_dtypes

import concourse.bass as bass
from concourse import bacc
import concourse.mybir as mybir
import concourse.tile as tile
from concourse import bass_utils

B, T, H = 128, 2048, 512
NCORES = 8
BPC = B // NCORES          # 16 batch rows per core
P = 128
HS = H // P                # 4 subtiles of the h/g dims
TC = 512                   # pass-1 t-chunk (max fp32 psum free dim)
NCH = T // TC              # 4 chunks
GROUP = 8                  # batch rows per softmax group
TP = T // P                # 16 t-subtiles for pass 2

f32 = mybir.dt.float32
f32r = mybir.dt.float32r
bf16 = mybir.dt.bfloat16
AF = mybir.ActivationFunctionType
AX = mybir.AxisListType

_CACHE = {}


def _build_nc():
    nc = bacc.Bacc(trn_type="TRN2", target_bir_lowering=False)

    keyT_bf = nc.dram_tensor("keyT_bf", [BPC, H, T], bf16, kind="ExternalInput")
    key_f32 = nc.dram_tensor("key_f32", [BPC, T, H], f32, kind="ExternalInput")
    w2T_bf = nc.dram_tensor("w2T_bf", [H, H], bf16, kind="ExternalInput")
    w1T = nc.dram_tensor("w1T", [H, H], f32, kind="ExternalInput")
    qT = nc.dram_tensor("qT", [H, BPC], f32, kind="ExternalInput")
    wout = nc.dram_tensor("wout", [H, 1], bf16, kind="ExternalInput")
    out = nc.dram_tensor("out", [BPC, H], f32, kind="ExternalOutput")

    with tile.TileContext(nc) as tc:
        with (
            tc.tile_pool(name="const", bufs=1) as cpool,
            tc.tile_pool(name="keyT", bufs=6) as keyT_pool,
            tc.tile_pool(name="keynat", bufs=8) as keynat_pool,
            tc.tile_pool(name="tanh", bufs=3) as tanh_pool,
            tc.tile_pool(name="sm", bufs=2) as sm_pool,
            tc.tile_pool(name="dram", bufs=2, space="DRAM") as dram_pool,
            tc.tile_pool(name="ps_kh", bufs=4, space="PSUM") as ps_kh,
            tc.tile_pool(name="ps_sc", bufs=2, space="PSUM") as ps_sc,
            tc.tile_pool(name="ps_o", bufs=2, space="PSUM") as ps_o,
        ):
            # ---- constants ----
            w2T_sb = cpool.tile([P, HS, H], bf16)      # [p, h_sub, g]
            nc.sync.dma_start(w2T_sb[:], w2T_bf.ap().rearrange("(s p) g -> p s g", p=P))
            w1T_sb = cpool.tile([P, HS, H], f32)
            nc.sync.dma_start(w1T_sb[:], w1T.ap().rearrange("(s p) g -> p s g", p=P))
            qT_sb = cpool.tile([P, HS, BPC], f32)
            nc.sync.dma_start(qT_sb[:], qT.ap().rearrange("(s p) b -> p s b", p=P))
            wout_sb = cpool.tile([P, HS], bf16)        # [p, g_sub]; lhsT slice [:, gs:gs+1]
            nc.sync.dma_start(wout_sb[:], wout.ap().rearrange("(s p) o -> p (s o)", p=P))
            qhT_sb = cpool.tile([P, HS, BPC], f32)     # relu(w1 q) with g on partitions

            # ---- q_h (transposed layout [g, b]) ----
            for gs in range(HS):
                ps_q = ps_kh.tile([P, TC], f32, tag="kh")
                for hs in range(HS):
                    nc.tensor.matmul(
                        ps_q[:, :BPC],
                        lhsT=w1T_sb[:, hs, gs * P:(gs + 1) * P],
                        rhs=qT_sb[:, hs, :],
                        start=(hs == 0),
                        stop=(hs == HS - 1),
                    )
                nc.scalar.activation(qhT_sb[:, gs, :], ps_q[:, :BPC], AF.Relu)

            # ---- main: groups of GROUP batch rows ----
            for g in range(BPC // GROUP):
                scores_sb = sm_pool.tile([GROUP, T], f32, tag="scores")

                # pass 1: scores
                for j in range(GROUP):
                    b = g * GROUP + j
                    keyT_b = keyT_bf.ap()[b].rearrange("(s p) t -> p s t", p=P)
                    stage = tanh_pool.tile([1, T], f32, tag="stage")
                    for c in range(NCH):
                        kt = keyT_pool.tile([P, HS, TC], bf16)
                        dma_eng = nc.sync if c % 2 == 0 else nc.scalar
                        dma_eng.dma_start(kt[:], keyT_b[:, :, c * TC:(c + 1) * TC])
                        ps_scores = ps_sc.tile([1, TC], f32)
                        for gs in range(HS):
                            ps_t = ps_kh.tile([P, TC], f32, tag="kh")
                            for hs in range(HS):
                                nc.tensor.matmul(
                                    ps_t[:],
                                    lhsT=w2T_sb[:, hs, gs * P:(gs + 1) * P],
                                    rhs=kt[:, hs, :],
                                    start=(hs == 0),
                                    stop=(hs == HS - 1),
                                )
                            # tanh(relu(k_h) + q_h): relu in-place on DVE,
                            # then tanh + per-partition q_h bias on ScalarE
                            rl = tanh_pool.tile([P, TC], f32, tag="rl")
                            nc.vector.tensor_scalar_max(rl[:], ps_t[:], 0.0)
                            th = tanh_pool.tile([P, TC], bf16)
                            nc.scalar.activation(
                                th[:], rl[:], AF.Tanh,
                                bias=qhT_sb[:, gs, b:b + 1],
                            )
                            nc.tensor.matmul(
                                ps_scores[:],
                                lhsT=wout_sb[:, gs:gs + 1],
                                rhs=th[:],
                                start=(gs == 0),
                                stop=(gs == HS - 1),
                            )
                        nc.vector.tensor_copy(
                            stage[:, c * TC:(c + 1) * TC], ps_scores[:]
                        )
                    nc.gpsimd.dma_start(scores_sb[j:j + 1, :], stage[:])

                # softmax over T, partition-parallel across the group
                mx = sm_pool.tile([GROUP, 1], f32, tag="mx")
                nc.vector.reduce_max(mx[:], scores_sb[:], axis=AX.X)
                nmx = sm_pool.tile([GROUP, 1], f32, tag="nmx")
                nc.vector.tensor_scalar_mul(nmx[:], mx[:], -1.0)
                expv = sm_pool.tile([GROUP, T], f32, tag="expv")
                sums = sm_pool.tile([GROUP, 1], f32, tag="sums")
                nc.scalar.activation(
                    expv[:], scores_sb[:], AF.Exp, bias=nmx[:], accum_out=sums[:]
                )
                inv = sm_pool.tile([GROUP, 1], f32, tag="inv")
                nc.vector.reciprocal(inv[:], sums[:])
                attn = sm_pool.tile([GROUP, T], f32, tag="attn")
                nc.vector.tensor_scalar_mul(attn[:], expv[:], inv[:])

                # transpose attn via DRAM round-trip: [GROUP, T] -> [P, TP*GROUP]
                attn_dram = dram_pool.tile([GROUP, T], f32)
                nc.gpsimd.dma_start(attn_dram[:], attn[:])
                attnT = sm_pool.tile([P, TP, GROUP], f32, tag="attnT")
                for j in range(GROUP):
                    nc.sync.dma_start(
                        attnT[:, :, j:j + 1],
                        attn_dram[j].rearrange("(c p) -> p c", p=P)[:, :, None],
                    )

                # pass 2: out = attn @ key (contract over t)
                for j in range(GROUP):
                    b = g * GROUP + j
                    ps_out = ps_o.tile([1, H], f32)
                    for c2 in range(TP):
                        kn = keynat_pool.tile([P, H], f32)
                        dma_eng2 = nc.sync if c2 % 2 == 0 else nc.scalar
                        dma_eng2.dma_start(kn[:], key_f32.ap()[b, c2 * P:(c2 + 1) * P, :])
                        nc.tensor.matmul(
                            ps_out[:],
                            lhsT=attnT[:, c2, j:j + 1],
                            rhs=kn[:],
                            start=(c2 == 0),
                            stop=(c2 == TP - 1),
                        )
                    ostage = tanh_pool.tile([1, H], f32, tag="ostage")
                    nc.vector.tensor_copy(ostage[:], ps_out[:])
                    nc.gpsimd.dma_start(out.ap()[b:b + 1, :], ostage[:])

    nc.compile()
    return nc


def kernel(query, key, w1, w2, w_out):
    query = np.asarray(query, dtype=np.float32)
    key = np.asarray(key, dtype=np.float32)
    w1 = np.asarray(w1, dtype=np.float32)
    w2 = np.asarray(w2, dtype=np.float32)
    w_out = np.asarray(w_out, dtype=np.float32)

    if "nc" not in _CACHE:
        _CACHE["nc"] = _build_nc()
    nc = _CACHE["nc"]

    w2T_bf = np.ascontiguousarray(w2.T).astype(ml_dtypes.bfloat16)
    w1T = np.ascontiguousarray(w1.T)
    wout_col = np.ascontiguousarray(w_out.reshape(H, 1)).astype(ml_dtypes.bfloat16)
    qT = np.ascontiguousarray(query.T)  # [H, B]

    in_maps = []
    for c in range(NCORES):
        sl = slice(c * BPC, (c + 1) * BPC)
        key_c = key[sl]
        in_maps.append({
            "keyT_bf": np.ascontiguousarray(
                key_c.transpose(0, 2, 1)).astype(ml_dtypes.bfloat16),
            "key_f32": np.ascontiguousarray(key_c),
            "w2T_bf": w2T_bf,
            "w1T": w1T,
            "qT": np.ascontiguousarray(qT[:, sl]),
            "wout": wout_col,
        })

    _CACHE["in_maps"] = in_maps
    import os
    trace = bool(int(os.environ.get("BENCH_TRACE", "0")))
    res = bass_utils.run_bass_kernel_spmd(
        nc, in_maps, core_ids=list(range(NCORES)), trace=trace)
    if trace:
        print(f"HW exec time: {res.exec_time_ns} ns")
        if res.instructions_and_trace:
            print("trace:", res.instructions_and_trace[1])
        _CACHE["res"] = res
    out = np.concatenate([r["out"] for r in res.results], axis=0)
    return out.astype(np.float32)
